# revision 4
# baseline (speedup 1.0000x reference)
"""Trainium2 Bass kernel for DirectConv2D (3x3 VALID, NCHW/OIHW).

Problem: x [32, 256, 56, 56] int32 (values 0..7 after clip),
         weight [256, 256, 3, 3] fp32 (small non-negative ints 0..6)
         -> out [32, 256, 54, 54] fp32.

Strategy:
 - Data-parallel across 8 NeuronCores: 4 images per core, weight replicated.
 - Conv decomposed into 9 shifted matmuls (one per kernel tap) accumulated
   in PSUM; contraction over the 256 input channels.
 - Inputs are tiny non-negative integers, so fp8-e4m3 matmuls are exact
   (products <= 42, fp32 PSUM accumulation). DoubleRow perf mode contracts
   all 256 input channels (2 x 128-partition k-tiles) per matmul.
 - Output computed in tiles of 9 rows x 56 cols; a strided moving AP keeps
   only the 54 valid cols (N=486 <= 512 fp32 PSUM bank).

Head (measured): NEFF preamble ends ~6.8us; dma_start doorbell-to-first-
packet is ~1.5us and early transfers are packet-rate bound, so packet SIZE
is everything. Image 0 is therefore shipped as six per-row-tile chunks,
each contiguous per partition (1240B lines), on the scalar ring, while the
sync ring carries the weights tap-major (2.5KB lines). ~4.7 bridge matmuls
(no data deps) keep the PE busy from the start barrier until the data
lands, so the HAM clock-gate window (~3.4-4.5us of *uninterrupted* busy at
1.2GHz before 2.4GHz unlocks) is never restarted; the real stream then
runs cold for its first ~2us. Any PE idle gap before HAM fires restarts
the window -- measured 107->110us for one 2.4us gap.

Tail: the very last tile's PSUM is evacuated in two row-aligned halves
stored on different rings; the other last-block tiles alternate rings so
both DMA queues are hot and drained when the final halves issue.
"""

import sys

sys.path.insert(0, "/opt/trn_rl_repo")

import ml_dtypes
import numpy as np

N_CORES = 8
IMGS = 4  # images per core
H = W = 56
OH = OW = 54
PIX = H * W  # 3136
PIXP = PIX + 4  # padded so kh=2,kw=2 window of width 504 stays in-bounds
ROWS_PER_TILE = 9
N_TILE = ROWS_PER_TILE * W  # 504 (<= 512 fp32 PSUM bank)
N_ROWTILES = OH // ROWS_PER_TILE  # 6
X0W = 620  # image-0 chunk width: 504 + max tap offset (114) + 2 slack

_PROGRAM_CACHE = {}


def _build_program(mode="fp8dr"):
    import concourse.bacc as bacc
    import concourse.mybir as mybir
    import concourse.tile as tile

    nc = bacc.Bacc(
        "TRN2",
        target_bir_lowering=False,
        debug=False,
        enable_asserts=False,
        num_devices=N_CORES,
    )
    dt8 = mybir.dt.float8e4
    dtb = mybir.dt.bfloat16
    dt_in = dt8 if mode == "fp8dr" else dtb

    # image 0 pre-chunked per row tile (contiguous 2*620B per partition),
    # weights tap-major (taps 0-4 / 5-8 slices are 2.5KB/partition lines)
    xh_d = nc.dram_tensor("x_head", [128, 6, 2, X0W], dt_in, kind="ExternalInput").ap()
    xr_d = nc.dram_tensor("x_rest", [128, 3, 2, PIXP], dt_in, kind="ExternalInput").ap()
    w_d = nc.dram_tensor("w_sb", [128, 9, 2, 2, 128], dt_in, kind="ExternalInput").ap()
    out_d = nc.dram_tensor(
        "out", [IMGS, 256, OH, OW], mybir.dt.float32, kind="ExternalOutput"
    ).ap()

    NT486 = ROWS_PER_TILE * OW  # 486 output pixels per row tile

    with tile.TileContext(nc) as tc:
        with (
            tc.tile_pool(name="const", bufs=1) as const_pool,
            tc.tile_pool(name="psum", bufs=8, space="PSUM") as psum_pool,
            tc.tile_pool(name="outs", bufs=3) as out_pool,
        ):
            # Bridge matmuls on scratch: keep the PE continuously busy from
            # the start barrier (~6.8us) until the first real data lands
            # (~9.3us) so the HAM busy window accumulates from the earliest
            # possible moment. Results are never read; in the fp8 build the
            # scratch stays uninitialized so the bridges have no deps beyond
            # a 2-byte memset. 4 x N486 + 1 x N270 cold ~= 2.7us.
            w_warm = const_pool.tile([128, 2, 128], dt_in)
            x_warm = const_pool.tile([128, 2, 544], dt_in)
            if mode != "fp8dr":
                nc.gpsimd.memset(w_warm, 0.0)
                nc.gpsimd.memset(x_warm, 0.0)
            else:
                nc.gpsimd.memset(w_warm[:, 0, 0:2], 0.0)
                nc.gpsimd.memset(x_warm[:, 0, 0:2], 0.0)
            pt_warm = psum_pool.tile([128, NT486], mybir.dt.float32, tag="pt")
            BRIDGE_ROWS = [9, 9, 9, 9, 5]
            for i, nr in enumerate(BRIDGE_ROWS):
                nb = nr * OW
                rhs_w = x_warm[:, :, 0 : nr * W].rearrange(
                    "p c (r q) -> p c r q", q=W
                )[:, :, :, 0:OW]
                if mode == "fp8dr":
                    nc.tensor.matmul(
                        pt_warm[:, 0:nb], w_warm, rhs_w,
                        start=(i == 0), stop=(i == len(BRIDGE_ROWS) - 1),
                        perf_mode=mybir.MatmulPerfMode.DoubleRow,
                    )
                else:
                    nc.tensor.matmul(
                        pt_warm[:, 0:nb], w_warm[:, 0], rhs_w[:, 0],
                        start=(i == 0), stop=(i == len(BRIDGE_ROWS) - 1),
                    )

            wt = const_pool.tile([128, 9, 2, 2, 128], dt_in)
            x0t = [
                const_pool.tile([128, 2, X0W], dt_in, name=f"x0t{t}", tag=f"x0t{t}")
                for t in range(6)
            ]
            xts = [None] + [
                const_pool.tile([128, 2, PIXP], dt_in, name=f"xt{n}", tag=f"xt{n}")
                for n in (1, 2, 3)
            ]
            # scalar ring: the six image-0 chunks in consumption order, then
            # images 2-3. sync ring: weights (taps 0-4 gate the first group,
            # taps 5-8 follow), then image 1. Early transfers are packet-
            # rate bound, so every head transfer is >=1.2KB/partition lines.
            for t in range(6):
                nc.scalar.dma_start(out=x0t[t], in_=xh_d[:, t])
            for n in (2, 3):
                for c in range(2):
                    nc.scalar.dma_start(out=xts[n][:, c], in_=xr_d[:, n - 1, c])
            nc.sync.dma_start(out=wt[:, 0:5], in_=w_d[:, 0:5])
            nc.sync.dma_start(out=wt[:, 5:9], in_=w_d[:, 5:9])
            for c in range(2):
                nc.sync.dma_start(out=xts[1][:, c], in_=xr_d[:, 0, c])

            def x_src(n, t):
                """(x tile, pixel base) holding rows needed by row tile t."""
                if n == 0:
                    return x0t[t], t * N_TILE
                return xts[n], 0

            for n in range(IMGS):
                for oc in range(2):
                    # staging for a full (n, oc) output block: dense 54x54
                    # rows so stores move 11.7KB-contiguous lines/partition.
                    ot = out_pool.tile([128, OH * OW], mybir.dt.float32)
                    for t in range(N_ROWTILES):
                        h0 = t * ROWS_PER_TILE
                        xsrc, xbase = x_src(n, t)
                        pt = psum_pool.tile([128, NT486], mybir.dt.float32)
                        k = 0
                        for kh in range(3):
                            for kw in range(3):
                                off = (h0 + kh) * W + kw - xbase
                                # strided moving AP skips the 2 junk cols per
                                # row: [128, 2, 9 rows (stride 56), 54 cols]
                                if mode == "fp8dr":
                                    rhs = xsrc[:, :, off : off + N_TILE].rearrange(
                                        "p c (r q) -> p c r q", q=W
                                    )[:, :, :, 0:OW]
                                    nc.tensor.matmul(
                                        pt,
                                        wt[:, k, oc],
                                        rhs,
                                        start=(k == 0),
                                        stop=(k == 8),
                                        perf_mode=mybir.MatmulPerfMode.DoubleRow,
                                    )
                                else:
                                    for c in range(2):
                                        rhs = xsrc[:, c, off : off + N_TILE].rearrange(
                                            "p (r q) -> p r q", q=W
                                        )[:, :, 0:OW]
                                        nc.tensor.matmul(
                                            pt,
                                            wt[:, k, oc, c],
                                            rhs,
                                            start=(k == 0 and c == 0),
                                            stop=(k == 8 and c == 1),
                                        )
                                k += 1
                        last_block = n == IMGS - 1 and oc == 1
                        if last_block and t == N_ROWTILES - 1:
                            # very last tile: split the PSUM evacuation into
                            # two row-aligned halves so the copy and the two
                            # half-stores (on different rings) overlap.
                            s = 5 * OW  # 5 rows + 4 rows
                            base = t * NT486
                            nc.vector.tensor_copy(
                                out=ot[:, base : base + s], in_=pt[:, 0:s]
                            )
                            nc.sync.dma_start(
                                out=out_d[n, oc * 128 : (oc + 1) * 128,
                                          h0 : h0 + 5, :],
                                in_=ot[:, base : base + s].rearrange(
                                    "p (h w) -> p h w", w=OW
                                ),
                            )
                            nc.vector.tensor_copy(
                                out=ot[:, base + s : base + NT486],
                                in_=pt[:, s:NT486],
                            )
                            nc.scalar.dma_start(
                                out=out_d[n, oc * 128 : (oc + 1) * 128,
                                          h0 + 5 : h0 + ROWS_PER_TILE, :],
                                in_=ot[:, base + s : base + NT486].rearrange(
                                    "p (h w) -> p h w", w=OW
                                ),
                            )
                        else:
                            nc.vector.tensor_copy(
                                out=ot[:, t * NT486 : (t + 1) * NT486], in_=pt
                            )
                        if last_block:
                            # fine-grained stores on the final block,
                            # alternating rings so both queues are hot and
                            # empty when the final halves issue: t0+t1 pair
                            # on sync, t2 on scalar, t3 on sync, t4 split
                            # across both. (t5 is the halves path above.)
                            if t == 1:
                                nc.sync.dma_start(
                                    out=out_d[n, oc * 128 : (oc + 1) * 128,
                                              0 : 2 * ROWS_PER_TILE, :],
                                    in_=ot[:, 0 : 2 * NT486].rearrange(
                                        "p (h w) -> p h w", w=OW
                                    ),
                                )
                            elif t == 2:
                                nc.scalar.dma_start(
                                    out=out_d[n, oc * 128 : (oc + 1) * 128,
                                              h0 : h0 + ROWS_PER_TILE, :],
                                    in_=ot[:, t * NT486 : (t + 1) * NT486].rearrange(
                                        "p (h w) -> p h w", w=OW
                                    ),
                                )
                            elif t == 3:
                                nc.sync.dma_start(
                                    out=out_d[n, oc * 128 : (oc + 1) * 128,
                                              h0 : h0 + ROWS_PER_TILE, :],
                                    in_=ot[:, t * NT486 : (t + 1) * NT486].rearrange(
                                        "p (h w) -> p h w", w=OW
                                    ),
                                )
                            elif t == 4:
                                s = 5 * OW
                                base = t * NT486
                                nc.sync.dma_start(
                                    out=out_d[n, oc * 128 : (oc + 1) * 128,
                                              h0 : h0 + 5, :],
                                    in_=ot[:, base : base + s].rearrange(
                                        "p (h w) -> p h w", w=OW
                                    ),
                                )
                                nc.scalar.dma_start(
                                    out=out_d[n, oc * 128 : (oc + 1) * 128,
                                              h0 + 5 : h0 + ROWS_PER_TILE, :],
                                    in_=ot[:, base + s : base + NT486].rearrange(
                                        "p (h w) -> p h w", w=OW
                                    ),
                                )
                    if not last_block:
                        nc.sync.dma_start(
                            out=out_d[n, oc * 128 : (oc + 1) * 128, :, :],
                            in_=ot.rearrange("p (h w) -> p h w", w=OW),
                        )
    nc.compile()
    return nc


def get_program(mode="fp8dr"):
    if mode not in _PROGRAM_CACHE:
        _PROGRAM_CACHE[mode] = _build_program(mode)
    return _PROGRAM_CACHE[mode]


def _np_dtype(mode):
    return ml_dtypes.float8_e4m3 if mode == "fp8dr" else ml_dtypes.bfloat16


def prep_weight(weight, mode="fp8dr"):
    """weight [256, 256, 3, 3] OIHW fp32 -> w_sb [128 ki, 9 tap, 2 oc, 2 c, 128 m]."""
    wq = weight.astype(np.int32).astype(np.float32)
    wq = wq.reshape(2, 128, 2, 128, 3, 3)  # [oc, m, c, ki, kh, kw]
    w_sb = np.ascontiguousarray(wq.transpose(3, 4, 5, 0, 2, 1))  # [ki, kh, kw, oc, c, m]
    w_sb = w_sb.reshape(128, 9, 2, 2, 128)
    return w_sb.astype(_np_dtype(mode))


def prep_x_core(x_core, mode="fp8dr"):
    """x_core [IMGS, 256, 56, 56] int32 ->
    x_head [128 ki, 6 t, 2 c, X0W] (image 0, per-row-tile chunks),
    x_rest [128 ki, 3 n, 2 c, PIXP] (images 1-3)."""
    xq = np.clip(x_core.astype(np.int32), 0, 7).astype(np.float32)
    xq = xq.reshape(IMGS, 2, 128, PIX)  # [n, c, ki, pix]
    xpad = np.zeros((IMGS, 2, 128, PIXP), np.float32)
    xpad[:, :, :, :PIX] = xq
    head = np.stack(
        [xpad[0, :, :, t * N_TILE : t * N_TILE + X0W] for t in range(6)], axis=0
    )  # [6, c, ki, X0W]
    x_head = np.ascontiguousarray(head.transpose(2, 0, 1, 3))  # [ki, t, c, X0W]
    x_rest = np.ascontiguousarray(xpad[1:].transpose(2, 0, 1, 3))  # [ki, n, c, PIXP]
    return x_head.astype(_np_dtype(mode)), x_rest.astype(_np_dtype(mode))


def make_in_maps(x, weight, mode="fp8dr"):
    w_sb = prep_weight(weight, mode)
    maps = []
    for c in range(N_CORES):
        x_head, x_rest = prep_x_core(x[c * IMGS : (c + 1) * IMGS], mode)
        maps.append({"x_head": x_head, "x_rest": x_rest, "w_sb": w_sb})
    return maps


def kernel(x, weight):
    import time

    from concourse.bass_utils import run_bass_kernel_spmd

    mode = "fp8dr"
    nc = get_program(mode)
    in_maps = make_in_maps(np.asarray(x), np.asarray(weight), mode)
    last_err = None
    for attempt in range(3):
        try:
            res = run_bass_kernel_spmd(nc, in_maps, list(range(N_CORES)))
            break
        except Exception as e:  # transient NRT_EXEC_UNIT_UNRECOVERABLE flakes
            last_err = e
            time.sleep(2.0)
    else:
        raise last_err
    return np.concatenate(
        [res.results[c]["out"] for c in range(N_CORES)], axis=0
    ).astype(np.float32)


# revision 6
# speedup vs baseline: 1.1712x; 1.1712x over previous
"""Trainium2 Bass kernel for DirectConv2D (3x3 VALID, NCHW/OIHW).

Problem: x [32, 256, 56, 56] int32 (values 0..7 after clip),
         weight [256, 256, 3, 3] fp32 (small non-negative ints 0..6)
         -> out [32, 256, 54, 54] fp32.

Strategy:
 - Data-parallel across 8 NeuronCores: 4 images per core, weight replicated.
 - Conv decomposed into 9 shifted matmuls (one per kernel tap) accumulated
   in PSUM; contraction over the 256 input channels.
 - Inputs are tiny non-negative integers, so fp8-e4m3 matmuls are exact
   (products <= 42, fp32 PSUM accumulation). DoubleRow perf mode contracts
   all 256 input channels (2 x 128-partition k-tiles) per matmul.
 - Output computed in tiles of 9 rows x 56 cols; a strided moving AP keeps
   only the 54 valid cols (N=486 <= 512 fp32 PSUM bank).

Head (measured): NEFF preamble ends ~6.8us; dma_start doorbell-to-first-
packet is ~1.5us and early transfers are packet-rate bound, so packet SIZE
is everything. Image 0 is therefore shipped as six per-row-tile chunks,
each contiguous per partition (1240B lines), on the scalar ring, while the
sync ring carries the weights tap-major (2.5KB lines). ~4.7 bridge matmuls
(no data deps) keep the PE busy from the start barrier until the data
lands, so the HAM clock-gate window (~3.4-4.5us of *uninterrupted* busy at
1.2GHz before 2.4GHz unlocks) is never restarted; the real stream then
runs cold for its first ~2us. Any PE idle gap before HAM fires restarts
the window -- measured 107->110us for one 2.4us gap.

Tail: the very last tile's PSUM is evacuated in two row-aligned halves
stored on different rings; the other last-block tiles alternate rings so
both DMA queues are hot and drained when the final halves issue.
"""

import sys

sys.path.insert(0, "/opt/trn_rl_repo")

import ml_dtypes
import numpy as np

N_CORES = 8
IMGS = 4  # images per core
H = W = 56
OH = OW = 54
PIX = H * W  # 3136
PIXP = PIX + 4  # padded so kh=2,kw=2 window of width 504 stays in-bounds
ROWS_PER_TILE = 9
N_TILE = ROWS_PER_TILE * W  # 504 (<= 512 fp32 PSUM bank)
N_ROWTILES = OH // ROWS_PER_TILE  # 6
X0W = 620  # image-0 chunk width: 504 + max tap offset (114) + 2 slack

_PROGRAM_CACHE = {}


def _build_program(mode="fp8dr"):
    import concourse.bacc as bacc
    import concourse.mybir as mybir
    import concourse.tile as tile

    nc = bacc.Bacc(
        "TRN2",
        target_bir_lowering=False,
        debug=False,
        enable_asserts=False,
        num_devices=N_CORES,
    )
    dt8 = mybir.dt.float8e4
    dtb = mybir.dt.bfloat16
    dt_in = dt8 if mode == "fp8dr" else dtb

    # image 0 pre-chunked per row tile (contiguous 2*620B per partition),
    # weights tap-major (taps 0-4 / 5-8 slices are 2.5KB/partition lines)
    xh_d = nc.dram_tensor("x_head", [128, 6, 2, X0W], dt_in, kind="ExternalInput").ap()
    xr_d = nc.dram_tensor("x_rest", [128, 3, 2, PIXP], dt_in, kind="ExternalInput").ap()
    w_d = nc.dram_tensor("w_sb", [128, 9, 2, 2, 128], dt_in, kind="ExternalInput").ap()
    out_d = nc.dram_tensor(
        "out", [IMGS, 256, OH, OW], mybir.dt.float32, kind="ExternalOutput"
    ).ap()

    NT486 = ROWS_PER_TILE * OW  # 486 output pixels per row tile

    with tile.TileContext(nc) as tc:
        with (
            tc.tile_pool(name="const", bufs=1) as const_pool,
            tc.tile_pool(name="psum", bufs=8, space="PSUM") as psum_pool,
            tc.tile_pool(name="outs", bufs=3) as out_pool,
        ):
            # Bridge matmuls on scratch: keep the PE continuously busy from
            # the start barrier (~6.8us) until the first real data lands
            # (~9.3us) so the HAM busy window accumulates from the earliest
            # possible moment. Results are never read; in the fp8 build the
            # scratch stays uninitialized so the bridges have no deps beyond
            # a 2-byte memset. 4 x N486 + 1 x N270 cold ~= 2.7us.
            w_warm = const_pool.tile([128, 2, 128], dt_in)
            x_warm = const_pool.tile([128, 2, 544], dt_in)
            if mode != "fp8dr":
                nc.gpsimd.memset(w_warm, 0.0)
                nc.gpsimd.memset(x_warm, 0.0)
            else:
                nc.gpsimd.memset(w_warm[:, 0, 0:2], 0.0)
                nc.gpsimd.memset(x_warm[:, 0, 0:2], 0.0)
            pt_warm = psum_pool.tile([128, NT486], mybir.dt.float32, tag="pt")
            BRIDGE_ROWS = [9, 9, 9, 9, 9]
            for i, nr in enumerate(BRIDGE_ROWS):
                nb = nr * OW
                rhs_w = x_warm[:, :, 0 : nr * W].rearrange(
                    "p c (r q) -> p c r q", q=W
                )[:, :, :, 0:OW]
                if mode == "fp8dr":
                    nc.tensor.matmul(
                        pt_warm[:, 0:nb], w_warm, rhs_w,
                        start=(i == 0), stop=(i == len(BRIDGE_ROWS) - 1),
                        perf_mode=mybir.MatmulPerfMode.DoubleRow,
                    )
                else:
                    nc.tensor.matmul(
                        pt_warm[:, 0:nb], w_warm[:, 0], rhs_w[:, 0],
                        start=(i == 0), stop=(i == len(BRIDGE_ROWS) - 1),
                    )

            wt = const_pool.tile([128, 9, 2, 2, 128], dt_in)
            # image 0: two small gating chunks (row tiles 0 and 1, each one
            # contiguous-per-partition transfer so the sem fires early) plus
            # one 4-chunk tile for row tiles 2-5. Images 1-3: one DMA each.
            # dma_start ISSUE costs ~0.8us on the sequencer and rings have a
            # small completion-sem pool, so keep the per-ring DMA count low.
            x0t0 = const_pool.tile([128, 2, X0W], dt_in, name="x0t0", tag="x0t0")
            x0t1 = const_pool.tile([128, 2, X0W], dt_in, name="x0t1", tag="x0t1")
            x0tl = const_pool.tile([128, 4, 2, X0W], dt_in, name="x0tl", tag="x0tl")
            xts = [None] + [
                const_pool.tile([128, 2, PIXP], dt_in, name=f"xt{n}", tag=f"xt{n}")
                for n in (1, 2, 3)
            ]
            # scalar ring: image-0 chunks in consumption order, then images
            # 2-3. sync ring: weights in tap order, then image 1.
            nc.scalar.dma_start(out=x0t0, in_=xh_d[:, 0])
            nc.scalar.dma_start(out=x0t1, in_=xh_d[:, 1])
            nc.scalar.dma_start(out=x0tl, in_=xh_d[:, 2:6])
            nc.scalar.dma_start(out=xts[2], in_=xr_d[:, 1])
            nc.scalar.dma_start(out=xts[3], in_=xr_d[:, 2])
            nc.sync.dma_start(out=wt[:, 0:2], in_=w_d[:, 0:2])
            nc.sync.dma_start(out=wt[:, 2:5], in_=w_d[:, 2:5])
            nc.sync.dma_start(out=wt[:, 5:9], in_=w_d[:, 5:9])
            nc.sync.dma_start(out=xts[1], in_=xr_d[:, 0])

            def x_src(n, t):
                """(x tile, pixel base) holding rows needed by row tile t."""
                if n == 0:
                    if t == 0:
                        return x0t0, 0
                    if t == 1:
                        return x0t1, N_TILE
                    return x0tl[:, t - 2], t * N_TILE
                return xts[n], 0

            for n in range(IMGS):
                for oc in range(2):
                    # staging for a full (n, oc) output block: dense 54x54
                    # rows so stores move 11.7KB-contiguous lines/partition.
                    ot = out_pool.tile([128, OH * OW], mybir.dt.float32)
                    for t in range(N_ROWTILES):
                        h0 = t * ROWS_PER_TILE
                        xsrc, xbase = x_src(n, t)
                        pt = psum_pool.tile([128, NT486], mybir.dt.float32)
                        k = 0
                        for kh in range(3):
                            for kw in range(3):
                                off = (h0 + kh) * W + kw - xbase
                                # strided moving AP skips the 2 junk cols per
                                # row: [128, 2, 9 rows (stride 56), 54 cols]
                                if mode == "fp8dr":
                                    rhs = xsrc[:, :, off : off + N_TILE].rearrange(
                                        "p c (r q) -> p c r q", q=W
                                    )[:, :, :, 0:OW]
                                    nc.tensor.matmul(
                                        pt,
                                        wt[:, k, oc],
                                        rhs,
                                        start=(k == 0),
                                        stop=(k == 8),
                                        perf_mode=mybir.MatmulPerfMode.DoubleRow,
                                    )
                                else:
                                    for c in range(2):
                                        rhs = xsrc[:, c, off : off + N_TILE].rearrange(
                                            "p (r q) -> p r q", q=W
                                        )[:, :, 0:OW]
                                        nc.tensor.matmul(
                                            pt,
                                            wt[:, k, oc, c],
                                            rhs,
                                            start=(k == 0 and c == 0),
                                            stop=(k == 8 and c == 1),
                                        )
                                k += 1
                        last_block = n == IMGS - 1 and oc == 1
                        if last_block and t == N_ROWTILES - 1:
                            # very last tile: split the PSUM evacuation into
                            # two row-aligned halves so the copy and the two
                            # half-stores (on different rings) overlap.
                            s = 5 * OW  # 5 rows + 4 rows
                            base = t * NT486
                            nc.vector.tensor_copy(
                                out=ot[:, base : base + s], in_=pt[:, 0:s]
                            )
                            nc.sync.dma_start(
                                out=out_d[n, oc * 128 : (oc + 1) * 128,
                                          h0 : h0 + 5, :],
                                in_=ot[:, base : base + s].rearrange(
                                    "p (h w) -> p h w", w=OW
                                ),
                            )
                            nc.vector.tensor_copy(
                                out=ot[:, base + s : base + NT486],
                                in_=pt[:, s:NT486],
                            )
                            nc.scalar.dma_start(
                                out=out_d[n, oc * 128 : (oc + 1) * 128,
                                          h0 + 5 : h0 + ROWS_PER_TILE, :],
                                in_=ot[:, base + s : base + NT486].rearrange(
                                    "p (h w) -> p h w", w=OW
                                ),
                            )
                        else:
                            nc.vector.tensor_copy(
                                out=ot[:, t * NT486 : (t + 1) * NT486], in_=pt
                            )
                        if last_block:
                            # fine-grained stores on the final block,
                            # alternating rings so both queues are hot and
                            # empty when the final halves issue: t0+t1 pair
                            # on sync, t2 on scalar, t3 on sync, t4 split
                            # across both. (t5 is the halves path above.)
                            if t == 1:
                                nc.sync.dma_start(
                                    out=out_d[n, oc * 128 : (oc + 1) * 128,
                                              0 : 2 * ROWS_PER_TILE, :],
                                    in_=ot[:, 0 : 2 * NT486].rearrange(
                                        "p (h w) -> p h w", w=OW
                                    ),
                                )
                            elif t == 2:
                                nc.scalar.dma_start(
                                    out=out_d[n, oc * 128 : (oc + 1) * 128,
                                              h0 : h0 + ROWS_PER_TILE, :],
                                    in_=ot[:, t * NT486 : (t + 1) * NT486].rearrange(
                                        "p (h w) -> p h w", w=OW
                                    ),
                                )
                            elif t == 3:
                                nc.sync.dma_start(
                                    out=out_d[n, oc * 128 : (oc + 1) * 128,
                                              h0 : h0 + ROWS_PER_TILE, :],
                                    in_=ot[:, t * NT486 : (t + 1) * NT486].rearrange(
                                        "p (h w) -> p h w", w=OW
                                    ),
                                )
                            elif t == 4:
                                s = 5 * OW
                                base = t * NT486
                                nc.sync.dma_start(
                                    out=out_d[n, oc * 128 : (oc + 1) * 128,
                                              h0 : h0 + 5, :],
                                    in_=ot[:, base : base + s].rearrange(
                                        "p (h w) -> p h w", w=OW
                                    ),
                                )
                                nc.scalar.dma_start(
                                    out=out_d[n, oc * 128 : (oc + 1) * 128,
                                              h0 + 5 : h0 + ROWS_PER_TILE, :],
                                    in_=ot[:, base + s : base + NT486].rearrange(
                                        "p (h w) -> p h w", w=OW
                                    ),
                                )
                    if not last_block:
                        nc.sync.dma_start(
                            out=out_d[n, oc * 128 : (oc + 1) * 128, :, :],
                            in_=ot.rearrange("p (h w) -> p h w", w=OW),
                        )
    nc.compile()
    return nc


def get_program(mode="fp8dr"):
    if mode not in _PROGRAM_CACHE:
        _PROGRAM_CACHE[mode] = _build_program(mode)
    return _PROGRAM_CACHE[mode]


def _np_dtype(mode):
    return ml_dtypes.float8_e4m3 if mode == "fp8dr" else ml_dtypes.bfloat16


def prep_weight(weight, mode="fp8dr"):
    """weight [256, 256, 3, 3] OIHW fp32 -> w_sb [128 ki, 9 tap, 2 oc, 2 c, 128 m]."""
    wq = weight.astype(np.int32).astype(np.float32)
    wq = wq.reshape(2, 128, 2, 128, 3, 3)  # [oc, m, c, ki, kh, kw]
    w_sb = np.ascontiguousarray(wq.transpose(3, 4, 5, 0, 2, 1))  # [ki, kh, kw, oc, c, m]
    w_sb = w_sb.reshape(128, 9, 2, 2, 128)
    return w_sb.astype(_np_dtype(mode))


def prep_x_core(x_core, mode="fp8dr"):
    """x_core [IMGS, 256, 56, 56] int32 ->
    x_head [128 ki, 6 t, 2 c, X0W] (image 0, per-row-tile chunks),
    x_rest [128 ki, 3 n, 2 c, PIXP] (images 1-3)."""
    xq = np.clip(x_core.astype(np.int32), 0, 7).astype(np.float32)
    xq = xq.reshape(IMGS, 2, 128, PIX)  # [n, c, ki, pix]
    xpad = np.zeros((IMGS, 2, 128, PIXP), np.float32)
    xpad[:, :, :, :PIX] = xq
    head = np.stack(
        [xpad[0, :, :, t * N_TILE : t * N_TILE + X0W] for t in range(6)], axis=0
    )  # [6, c, ki, X0W]
    x_head = np.ascontiguousarray(head.transpose(2, 0, 1, 3))  # [ki, t, c, X0W]
    x_rest = np.ascontiguousarray(xpad[1:].transpose(2, 0, 1, 3))  # [ki, n, c, PIXP]
    return x_head.astype(_np_dtype(mode)), x_rest.astype(_np_dtype(mode))


def make_in_maps(x, weight, mode="fp8dr"):
    w_sb = prep_weight(weight, mode)
    maps = []
    for c in range(N_CORES):
        x_head, x_rest = prep_x_core(x[c * IMGS : (c + 1) * IMGS], mode)
        maps.append({"x_head": x_head, "x_rest": x_rest, "w_sb": w_sb})
    return maps


def kernel(x, weight):
    import time

    from concourse.bass_utils import run_bass_kernel_spmd

    mode = "fp8dr"
    nc = get_program(mode)
    in_maps = make_in_maps(np.asarray(x), np.asarray(weight), mode)
    last_err = None
    for attempt in range(3):
        try:
            res = run_bass_kernel_spmd(nc, in_maps, list(range(N_CORES)))
            break
        except Exception as e:  # transient NRT_EXEC_UNIT_UNRECOVERABLE flakes
            last_err = e
            time.sleep(2.0)
    else:
        raise last_err
    return np.concatenate(
        [res.results[c]["out"] for c in range(N_CORES)], axis=0
    ).astype(np.float32)


# revision 10
# speedup vs baseline: 1.1737x; 1.0021x over previous
"""Trainium2 Bass kernel for DirectConv2D (3x3 VALID, NCHW/OIHW).

Problem: x [32, 256, 56, 56] int32 (values 0..7 after clip),
         weight [256, 256, 3, 3] fp32 (small non-negative ints 0..6)
         -> out [32, 256, 54, 54] fp32.

Strategy:
 - Data-parallel across 8 NeuronCores: 4 images per core, weight replicated.
 - Conv decomposed into 9 shifted matmuls (one per kernel tap) accumulated
   in PSUM; contraction over the 256 input channels.
 - Inputs are tiny non-negative integers, so fp8-e4m3 matmuls are exact
   (products <= 42, fp32 PSUM accumulation). DoubleRow perf mode contracts
   all 256 input channels (2 x 128-partition k-tiles) per matmul.
 - Output computed in tiles of 9 rows x 56 cols; a strided moving AP keeps
   only the 54 valid cols (N=486 <= 512 fp32 PSUM bank).

Head (measured): NEFF preamble ends ~6.8us; dma_start doorbell-to-first-
packet is ~1.5us and early transfers are packet-rate bound, so packet SIZE
is everything. Image 0 is therefore shipped as six per-row-tile chunks,
each contiguous per partition (1240B lines), on the scalar ring, while the
sync ring carries the weights tap-major (2.5KB lines). ~4.7 bridge matmuls
(no data deps) keep the PE busy from the start barrier until the data
lands, so the HAM clock-gate window (~3.4-4.5us of *uninterrupted* busy at
1.2GHz before 2.4GHz unlocks) is never restarted; the real stream then
runs cold for its first ~2us. Any PE idle gap before HAM fires restarts
the window -- measured 107->110us for one 2.4us gap.

Tail: the very last tile's PSUM is evacuated in two row-aligned halves
stored on different rings; the other last-block tiles alternate rings so
both DMA queues are hot and drained when the final halves issue.
"""

import sys

sys.path.insert(0, "/opt/trn_rl_repo")

import ml_dtypes
import numpy as np

N_CORES = 8
IMGS = 4  # images per core
H = W = 56
OH = OW = 54
PIX = H * W  # 3136
PIXP = PIX + 4  # padded so kh=2,kw=2 window of width 504 stays in-bounds
ROWS_PER_TILE = 9
N_TILE = ROWS_PER_TILE * W  # 504 (<= 512 fp32 PSUM bank)
N_ROWTILES = OH // ROWS_PER_TILE  # 6
X0W = 620  # image-0 chunk width: 504 + max tap offset (114) + 2 slack

_PROGRAM_CACHE = {}


def _build_program(mode="fp8dr"):
    import concourse.bacc as bacc
    import concourse.mybir as mybir
    import concourse.tile as tile

    nc = bacc.Bacc(
        "TRN2",
        target_bir_lowering=False,
        debug=False,
        enable_asserts=False,
        num_devices=N_CORES,
    )
    dt8 = mybir.dt.float8e4
    dtb = mybir.dt.bfloat16
    dt_in = dt8 if mode == "fp8dr" else dtb

    # image 0 pre-chunked per row tile (contiguous 2*620B per partition),
    # weights tap-major (taps 0-4 / 5-8 slices are 2.5KB/partition lines)
    xh_d = nc.dram_tensor("x_head", [128, 6, 2, X0W], dt_in, kind="ExternalInput").ap()
    xr_d = nc.dram_tensor("x_rest", [128, 3, 2, PIXP], dt_in, kind="ExternalInput").ap()
    w_d = nc.dram_tensor("w_sb", [128, 9, 2, 2, 128], dt_in, kind="ExternalInput").ap()
    out_d = nc.dram_tensor(
        "out", [IMGS, 256, OH, OW], mybir.dt.float32, kind="ExternalOutput"
    ).ap()

    NT486 = ROWS_PER_TILE * OW  # 486 output pixels per row tile

    with tile.TileContext(nc) as tc:
        with (
            tc.tile_pool(name="const", bufs=1) as const_pool,
            tc.tile_pool(name="psum", bufs=8, space="PSUM") as psum_pool,
            tc.tile_pool(name="outs", bufs=3) as out_pool,
        ):
            # Bridge matmuls on scratch: keep the PE continuously busy from
            # the start barrier (~6.8us) until the first real data lands
            # (~9.3us) so the HAM busy window accumulates from the earliest
            # possible moment. Results are never read; in the fp8 build the
            # scratch stays uninitialized so the bridges have no deps beyond
            # a 2-byte memset. 4 x N486 + 1 x N270 cold ~= 2.7us.
            w_warm = const_pool.tile([128, 2, 128], dt_in)
            x_warm = const_pool.tile([128, 2, 544], dt_in)
            if mode != "fp8dr":
                nc.gpsimd.memset(w_warm, 0.0)
                nc.gpsimd.memset(x_warm, 0.0)
            else:
                nc.gpsimd.memset(w_warm[:, 0, 0:2], 0.0)
                nc.gpsimd.memset(x_warm[:, 0, 0:2], 0.0)
            pt_warm = psum_pool.tile([128, NT486], mybir.dt.float32, tag="pt")
            # 8 bridges ~= 3.6us: cover the measured data-landing time
            # (doorbell ~1.4us after the barrier + ~2us aggregate transfer +
            # ~1us 16-way completion-sem tail) with margin, since a PE idle
            # seam before HAM fires restarts the 3.4us busy window.
            BRIDGE_ROWS = [9] * 8
            for i, nr in enumerate(BRIDGE_ROWS):
                nb = nr * OW
                rhs_w = x_warm[:, :, 0 : nr * W].rearrange(
                    "p c (r q) -> p c r q", q=W
                )[:, :, :, 0:OW]
                if mode == "fp8dr":
                    nc.tensor.matmul(
                        pt_warm[:, 0:nb], w_warm, rhs_w,
                        start=(i == 0), stop=(i == len(BRIDGE_ROWS) - 1),
                        perf_mode=mybir.MatmulPerfMode.DoubleRow,
                    )
                else:
                    nc.tensor.matmul(
                        pt_warm[:, 0:nb], w_warm[:, 0], rhs_w[:, 0],
                        start=(i == 0), stop=(i == len(BRIDGE_ROWS) - 1),
                    )

            wt = const_pool.tile([128, 9, 2, 2, 128], dt_in)
            # image 0: two small gating chunks (row tiles 0 and 1, each one
            # contiguous-per-partition transfer so the sem fires early) plus
            # one 4-chunk tile for row tiles 2-5. Images 1-3: one DMA each.
            # dma_start ISSUE costs ~0.8us on the sequencer and rings have a
            # small completion-sem pool, so keep the per-ring DMA count low.
            x0ab = const_pool.tile([128, 2, 2, X0W], dt_in, name="x0ab", tag="x0ab")
            x0tl = const_pool.tile([128, 4, 2, X0W], dt_in, name="x0tl", tag="x0tl")
            xts = [None] + [
                const_pool.tile([128, 2, PIXP], dt_in, name=f"xt{n}", tag=f"xt{n}")
                for n in (1, 2, 3)
            ]
            # scalar ring: image-0 chunks in consumption order, then images
            # 2-3. sync ring: weights in tap order, then image 1. Row tiles
            # 0 AND 1 ship as one transfer: the scheduler hoists row tile
            # 1's first matmul directly behind row tile 0's, so both must
            # land together or the PE FIFO stalls on the hoisted matmul.
            nc.scalar.dma_start(out=x0ab, in_=xh_d[:, 0:2])
            nc.scalar.dma_start(out=x0tl, in_=xh_d[:, 2:6])
            nc.scalar.dma_start(out=xts[2], in_=xr_d[:, 1])
            nc.scalar.dma_start(out=xts[3], in_=xr_d[:, 2])
            nc.sync.dma_start(out=wt[:, 0:2], in_=w_d[:, 0:2])
            nc.sync.dma_start(out=wt[:, 2:5], in_=w_d[:, 2:5])
            nc.sync.dma_start(out=wt[:, 5:9], in_=w_d[:, 5:9])
            nc.sync.dma_start(out=xts[1], in_=xr_d[:, 0])

            def x_src(n, t):
                """(x tile, pixel base) holding rows needed by row tile t."""
                if n == 0:
                    if t < 2:
                        return x0ab[:, t], t * N_TILE
                    return x0tl[:, t - 2], t * N_TILE
                return xts[n], 0

            for n in range(IMGS):
                for oc in range(2):
                    # staging for a full (n, oc) output block: dense 54x54
                    # rows so stores move 11.7KB-contiguous lines/partition.
                    ot = out_pool.tile([128, OH * OW], mybir.dt.float32)
                    for t in range(N_ROWTILES):
                        h0 = t * ROWS_PER_TILE
                        xsrc, xbase = x_src(n, t)
                        pt = psum_pool.tile([128, NT486], mybir.dt.float32)
                        k = 0
                        for kh in range(3):
                            for kw in range(3):
                                off = (h0 + kh) * W + kw - xbase
                                # strided moving AP skips the 2 junk cols per
                                # row: [128, 2, 9 rows (stride 56), 54 cols]
                                if mode == "fp8dr":
                                    rhs = xsrc[:, :, off : off + N_TILE].rearrange(
                                        "p c (r q) -> p c r q", q=W
                                    )[:, :, :, 0:OW]
                                    nc.tensor.matmul(
                                        pt,
                                        wt[:, k, oc],
                                        rhs,
                                        start=(k == 0),
                                        stop=(k == 8),
                                        perf_mode=mybir.MatmulPerfMode.DoubleRow,
                                    )
                                else:
                                    for c in range(2):
                                        rhs = xsrc[:, c, off : off + N_TILE].rearrange(
                                            "p (r q) -> p r q", q=W
                                        )[:, :, 0:OW]
                                        nc.tensor.matmul(
                                            pt,
                                            wt[:, k, oc, c],
                                            rhs,
                                            start=(k == 0 and c == 0),
                                            stop=(k == 8 and c == 1),
                                        )
                                k += 1
                        last_block = n == IMGS - 1 and oc == 1
                        if last_block and t == N_ROWTILES - 1:
                            # very last tile: evacuate the PSUM in two row-
                            # aligned halves on two DIFFERENT engines (ACT +
                            # DVE) so the copies run in parallel, and store
                            # the halves on different rings.
                            s = 5 * OW  # 5 rows + 4 rows
                            base = t * NT486
                            nc.scalar.copy(
                                out=ot[:, base : base + s], in_=pt[:, 0:s]
                            )
                            nc.sync.dma_start(
                                out=out_d[n, oc * 128 : (oc + 1) * 128,
                                          h0 : h0 + 5, :],
                                in_=ot[:, base : base + s].rearrange(
                                    "p (h w) -> p h w", w=OW
                                ),
                            )
                            nc.vector.tensor_copy(
                                out=ot[:, base + s : base + NT486],
                                in_=pt[:, s:NT486],
                            )
                            nc.scalar.dma_start(
                                out=out_d[n, oc * 128 : (oc + 1) * 128,
                                          h0 + 5 : h0 + ROWS_PER_TILE, :],
                                in_=ot[:, base + s : base + NT486].rearrange(
                                    "p (h w) -> p h w", w=OW
                                ),
                            )
                        else:
                            nc.vector.tensor_copy(
                                out=ot[:, t * NT486 : (t + 1) * NT486], in_=pt
                            )
                        if last_block:
                            # fine-grained stores on the final block: t0+t1
                            # pair, t2 and t4 on sync; t3 on scalar so the
                            # scalar ring is idle (prompt issue) when the
                            # final half lands on it. (t5 is above.)
                            if t == 1:
                                nc.sync.dma_start(
                                    out=out_d[n, oc * 128 : (oc + 1) * 128,
                                              0 : 2 * ROWS_PER_TILE, :],
                                    in_=ot[:, 0 : 2 * NT486].rearrange(
                                        "p (h w) -> p h w", w=OW
                                    ),
                                )
                            elif t == 3:
                                nc.scalar.dma_start(
                                    out=out_d[n, oc * 128 : (oc + 1) * 128,
                                              h0 : h0 + ROWS_PER_TILE, :],
                                    in_=ot[:, t * NT486 : (t + 1) * NT486].rearrange(
                                        "p (h w) -> p h w", w=OW
                                    ),
                                )
                            elif t in (2, 4):
                                nc.sync.dma_start(
                                    out=out_d[n, oc * 128 : (oc + 1) * 128,
                                              h0 : h0 + ROWS_PER_TILE, :],
                                    in_=ot[:, t * NT486 : (t + 1) * NT486].rearrange(
                                        "p (h w) -> p h w", w=OW
                                    ),
                                )
                    if not last_block:
                        nc.sync.dma_start(
                            out=out_d[n, oc * 128 : (oc + 1) * 128, :, :],
                            in_=ot.rearrange("p (h w) -> p h w", w=OW),
                        )
    nc.compile()
    return nc


def get_program(mode="fp8dr"):
    if mode not in _PROGRAM_CACHE:
        _PROGRAM_CACHE[mode] = _build_program(mode)
    return _PROGRAM_CACHE[mode]


def _np_dtype(mode):
    return ml_dtypes.float8_e4m3 if mode == "fp8dr" else ml_dtypes.bfloat16


def prep_weight(weight, mode="fp8dr"):
    """weight [256, 256, 3, 3] OIHW fp32 -> w_sb [128 ki, 9 tap, 2 oc, 2 c, 128 m]."""
    wq = weight.astype(np.int32).astype(np.float32)
    wq = wq.reshape(2, 128, 2, 128, 3, 3)  # [oc, m, c, ki, kh, kw]
    w_sb = np.ascontiguousarray(wq.transpose(3, 4, 5, 0, 2, 1))  # [ki, kh, kw, oc, c, m]
    w_sb = w_sb.reshape(128, 9, 2, 2, 128)
    return w_sb.astype(_np_dtype(mode))


def prep_x_core(x_core, mode="fp8dr"):
    """x_core [IMGS, 256, 56, 56] int32 ->
    x_head [128 ki, 6 t, 2 c, X0W] (image 0, per-row-tile chunks),
    x_rest [128 ki, 3 n, 2 c, PIXP] (images 1-3)."""
    xq = np.clip(x_core.astype(np.int32), 0, 7).astype(np.float32)
    xq = xq.reshape(IMGS, 2, 128, PIX)  # [n, c, ki, pix]
    xpad = np.zeros((IMGS, 2, 128, PIXP), np.float32)
    xpad[:, :, :, :PIX] = xq
    head = np.stack(
        [xpad[0, :, :, t * N_TILE : t * N_TILE + X0W] for t in range(6)], axis=0
    )  # [6, c, ki, X0W]
    x_head = np.ascontiguousarray(head.transpose(2, 0, 1, 3))  # [ki, t, c, X0W]
    x_rest = np.ascontiguousarray(xpad[1:].transpose(2, 0, 1, 3))  # [ki, n, c, PIXP]
    return x_head.astype(_np_dtype(mode)), x_rest.astype(_np_dtype(mode))


def make_in_maps(x, weight, mode="fp8dr"):
    w_sb = prep_weight(weight, mode)
    maps = []
    for c in range(N_CORES):
        x_head, x_rest = prep_x_core(x[c * IMGS : (c + 1) * IMGS], mode)
        maps.append({"x_head": x_head, "x_rest": x_rest, "w_sb": w_sb})
    return maps


def kernel(x, weight):
    import time

    from concourse.bass_utils import run_bass_kernel_spmd

    mode = "fp8dr"
    nc = get_program(mode)
    in_maps = make_in_maps(np.asarray(x), np.asarray(weight), mode)
    last_err = None
    for attempt in range(3):
        try:
            res = run_bass_kernel_spmd(nc, in_maps, list(range(N_CORES)))
            break
        except Exception as e:  # transient NRT_EXEC_UNIT_UNRECOVERABLE flakes
            last_err = e
            time.sleep(2.0)
    else:
        raise last_err
    return np.concatenate(
        [res.results[c]["out"] for c in range(N_CORES)], axis=0
    ).astype(np.float32)
